# revision 10
# baseline (speedup 1.0000x reference)
"""Trainium2 Bass kernel for nn_ConvBlock (1x1-conv attention block).

Reference computation (per (b,h) "pair", batched over B*H = 1024 pairs):
    f = x @ Wf + bf          [W, Ck]   (Ck = 32)
    g = x @ Wg + bg          [W, Ck]
    h = x @ Wh + bh          [W, C]
    scores = f @ g.T         [W, W]
    attn   = sigmoid(scores)
    out    = attn @ h        [W, C]

Sharding: data-parallel over the 1024 (b,h) pairs, 128 pairs per core on
8 NeuronCores; the small 1x1-conv weights are replicated.

Per-core kernel layout: the host pre-transposes x so each pair's slice
arrives as xT[c, w] packed into one [128, 512] SBUF tile
(partition = c % 128, free = (c // 128) * 256 + w).  All matmuls then run
in the PE's native orientation with no on-chip transposes:
    fgT  = [Wf|Wg].T @ xT        [64, 256]   (PSUM, K=256 in 2 chunks)
    h    = xT.T @ Wh + 1 x bh    [256, 256]  (two 128-row halves of one bank)
    sT   = gT.T @ fT             [256, 256]  (scores^T, two halves)
    aT   = sigmoid(sT)           (ACT, PSUM->SBUF)
    out  = aT.T @ h              [256, 256]  (two halves)
"""

import os
import sys
import types
from contextlib import ExitStack

sys.path.insert(0, "/opt/trn_rl_repo")
import numpy as np

B, H, W, C = 4, 256, 256, 256
CK = C // 8                      # 32
N_CORES = 8
PAIRS = B * H                    # 1024
PPC = PAIRS // N_CORES           # 128 pairs per core

# matmul input dtype: "f32r" (full-rate fp32), "f32" (quarter-rate), "bf16"
MM_DT = os.environ.get("KBENCH_MM_DT", "f32r")

_compiled = {}


def _install_ntff_hook_shim():
    """run_bass_kernel_spmd(trace=True) under axon looks up an NTFF profile
    hook via antenv.axon_hooks, which this image's antenv lacks. Register the
    ctypes-based hook from trn_agent_boot under that name."""
    import antenv

    if "antenv.axon_hooks" in sys.modules:
        return
    mod = types.ModuleType("antenv.axon_hooks")
    state = {"hook": None}
    mod.set_axon_ntff_profile_hook = lambda h: state.__setitem__("hook", h)
    mod.get_axon_ntff_profile_hook = lambda: state["hook"]
    sys.modules["antenv.axon_hooks"] = mod
    antenv.axon_hooks = mod
    from trn_agent_boot.trn_boot import _ntff_profile_via_ctypes

    mod.set_axon_ntff_profile_hook(
        _ntff_profile_via_ctypes("/opt/axon/libaxon_pjrt.so")
    )


def _build_nc(mm_dt: str):
    import concourse.bacc as bacc
    import concourse.mybir as mybir
    import concourse.tile as tile

    f32 = mybir.dt.float32
    bf16 = mybir.dt.bfloat16
    use_bf16 = mm_dt == "bf16"
    # dtype for every matmul input (tiles and the x/weight DRAM params):
    # float32r is fp32-layout fed to the PE at full rate; engines round on
    # write, and the host bytes pass through unchanged.
    if mm_dt == "f32r":
        wdt = mybir.dt.float32r
    elif mm_dt == "bf16":
        wdt = bf16
    else:
        wdt = f32

    def cc(ap):
        return ap

    Identity = mybir.ActivationFunctionType.Identity
    Sigmoid = mybir.ActivationFunctionType.Sigmoid

    nc = bacc.Bacc("TRN2", target_bir_lowering=False, debug=False)

    xdt = f32 if use_bf16 else wdt
    xt_d = nc.declare_dram_parameter("xt", [PPC, 128, 512], xdt, isOutput=False)
    wfg_d = nc.declare_dram_parameter("wfg", [2, 128, 2 * CK], xdt, isOutput=False)
    wh_d = nc.declare_dram_parameter("wh", [2, 128, 256], xdt, isOutput=False)
    bfg_d = nc.declare_dram_parameter("bfg", [CK, 2], f32, isOutput=False)
    bhb_d = nc.declare_dram_parameter("bhb", [128, 512], f32, isOutput=False)
    out_d = nc.declare_dram_parameter("out", [PPC, 128, 512], f32, isOutput=True)

    with tile.TileContext(nc) as tc, ExitStack() as ctx:
        const = ctx.enter_context(tc.tile_pool(name="const", bufs=1))
        data = ctx.enter_context(tc.tile_pool(name="data", bufs=4))
        psum = ctx.enter_context(tc.tile_pool(name="psum", bufs=2, space="PSUM"))

        # constants: partition dim first, K-chunk in the free dims
        wfg = const.tile([128, 2, 2 * CK], wdt)     # [c%128, c//128, 2Ck]
        wh = const.tile([128, 2, 256], wdt)         # [c%128, c//128, o]
        bfg = const.tile([CK, 2], f32)              # [:, 0]=bf, [:, 1]=bg
        bhb = const.tile([128, 512], f32)           # bh broadcast over partitions
        if not use_bf16:
            nc.sync.dma_start(wfg[:], wfg_d[:].rearrange("k p n -> p k n"))
            nc.sync.dma_start(wh[:], wh_d[:].rearrange("k p n -> p k n"))
        else:
            wfg_f = const.tile([128, 2, 2 * CK], f32, tag="wfg_f")
            wh_f = const.tile([128, 2, 256], f32, tag="wh_f")
            nc.sync.dma_start(wfg_f[:], wfg_d[:].rearrange("k p n -> p k n"))
            nc.sync.dma_start(wh_f[:], wh_d[:].rearrange("k p n -> p k n"))
            nc.vector.tensor_copy(wfg[:], wfg_f[:])
            nc.vector.tensor_copy(wh[:], wh_f[:])
        nc.sync.dma_start(bfg[:], bfg_d[:])
        nc.sync.dma_start(bhb[:], bhb_d[:])

        fgs, hs, ats = {}, {}, {}

        def stage_a(i):
            # load + projections: fgT (PSUM->SBUF w/ bias) and h (+bh)
            xt = data.tile([128, 512], xdt, tag="xt")
            nc.sync.dma_start(xt[:], xt_d[i])
            if use_bf16:
                xtb = data.tile([128, 512], bf16, tag="xtb")
                nc.vector.tensor_copy(xtb[:], xt[:])
                xm = xtb
            else:
                xm = xt

            # f -> fg_ps[:, 0:256], g -> fg_ps[:, 256:512]; both base partition 0
            fg_ps = psum.tile([CK, 512], f32, tag="fg_ps")
            for k in range(2):
                nc.tensor.matmul(
                    fg_ps[:, 0:256], cc(wfg[:, k, 0:CK]),
                    cc(xm[:, 256 * k : 256 * (k + 1)]),
                    start=(k == 0), stop=(k == 1),
                )
            for k in range(2):
                nc.tensor.matmul(
                    fg_ps[:, 256:512], cc(wfg[:, k, CK : 2 * CK]),
                    cc(xm[:, 256 * k : 256 * (k + 1)]),
                    start=(k == 0), stop=(k == 1),
                )
            fg = data.tile([CK, 512], wdt, tag="fg")
            nc.scalar.activation(
                fg[:, 0:256], fg_ps[:, 0:256], Identity, bias=bfg[:, 0:1]
            )
            nc.scalar.activation(
                fg[:, 256:512], fg_ps[:, 256:512], Identity, bias=bfg[:, 1:2]
            )
            fgs[i] = fg

            h_ps = psum.tile([128, 512], f32, tag="h_ps")
            for wc in range(2):
                half = h_ps[:, 256 * wc : 256 * (wc + 1)]
                nc.tensor.matmul(
                    half, cc(xm[:, 128 * wc : 128 * (wc + 1)]), cc(wh[:, 0]),
                    start=True, stop=False,
                )
                nc.tensor.matmul(
                    half, cc(xm[:, 256 + 128 * wc : 256 + 128 * (wc + 1)]),
                    cc(wh[:, 1]),
                    start=False, stop=True,
                )
            h = data.tile([128, 512], wdt, tag="h")
            nc.vector.tensor_add(h[:], h_ps[:], bhb[:])
            hs[i] = h

        def stage_b(i):
            # scores^T = gT.T @ fT, then sigmoid
            fg = fgs.pop(i)
            sT_ps = psum.tile([128, 512], f32, tag="sT_ps")
            nc.tensor.matmul(
                sT_ps[:, 0:256], cc(fg[:, 256:384]), cc(fg[:, 0:256]),
                start=True, stop=True,
            )
            nc.tensor.matmul(
                sT_ps[:, 256:512], cc(fg[:, 384:512]), cc(fg[:, 0:256]),
                start=True, stop=True,
            )
            aT = data.tile([128, 512], wdt, tag="aT")
            nc.scalar.activation(aT[:], sT_ps[:], Sigmoid)
            ats[i] = aT

        def stage_c(i):
            # out = attn @ h, copy out, store
            aT = ats.pop(i)
            h = hs.pop(i)
            o_ps = psum.tile([128, 512], f32, tag="o_ps")
            for ic in range(2):
                half = o_ps[:, 256 * ic : 256 * (ic + 1)]
                nc.tensor.matmul(
                    half, cc(aT[:, 128 * ic : 128 * (ic + 1)]), cc(h[:, 0:256]),
                    start=True, stop=False,
                )
                nc.tensor.matmul(
                    half, cc(aT[:, 256 + 128 * ic : 256 + 128 * (ic + 1)]),
                    cc(h[:, 256:512]),
                    start=False, stop=True,
                )
            ot = data.tile([128, 512], f32, tag="ot")
            nc.vector.tensor_copy(ot[:], o_ps[:])
            nc.sync.dma_start(out_d[i], ot[:])

        # software pipeline across pairs so PE never waits on ACT results
        for i in range(PPC + 2):
            if i < PPC:
                stage_a(i)
            if 1 <= i <= PPC:
                stage_b(i - 1)
            if i >= 2:
                stage_c(i - 2)

    nc.compile()
    return nc


def _get_nc(mm_dt: str):
    if mm_dt not in _compiled:
        _compiled[mm_dt] = _build_nc(mm_dt)
    return _compiled[mm_dt]


def _prep_in_maps(x, Wf, bf, Wg, bg, Wh, bh):
    x = np.ascontiguousarray(np.asarray(x, dtype=np.float32))
    # [pair, w, c] -> [pair, c%128, (c//128)*256 + w]
    xt = (
        x.reshape(PAIRS, W, 2, 128)
        .transpose(0, 3, 2, 1)
        .reshape(PAIRS, 128, 512)
    )
    wfg = np.concatenate(
        [np.asarray(Wf, np.float32), np.asarray(Wg, np.float32)], axis=1
    ).reshape(2, 128, 2 * CK)
    whr = np.asarray(Wh, np.float32).reshape(2, 128, 256)
    bfg = np.ascontiguousarray(
        np.stack(
            [np.asarray(bf, np.float32), np.asarray(bg, np.float32)], axis=1
        )
    )  # [CK, 2]: [:, 0] = bf, [:, 1] = bg
    bhb = np.ascontiguousarray(
        np.tile(np.asarray(bh, np.float32), (128, 2))
    )  # [128, 512], [p, k*256+o] = bh[o]
    in_maps = []
    for c in range(N_CORES):
        shard = np.ascontiguousarray(xt[c * PPC : (c + 1) * PPC])
        in_maps.append(
            {"xt": shard, "wfg": wfg, "wh": whr, "bfg": bfg, "bhb": bhb}
        )
    return in_maps


def _unprep_output(results):
    out = np.concatenate([r["out"] for r in results], axis=0)  # [1024,128,512]
    # [pair, m, kc*256+o] -> [pair, kc*128+m, o]
    out = (
        out.reshape(PAIRS, 128, 2, 256)
        .transpose(0, 2, 1, 3)
        .reshape(B, H, W, C)
    )
    return np.ascontiguousarray(out)


def _run(inputs, trace=False, tmpdir=None):
    from concourse import bass_utils
    from concourse.bass_utils import run_bass_kernel_spmd

    if trace:
        _install_ntff_hook_shim()
        bass_utils.upload_artifacts = lambda d: "local://" + d

    nc = _get_nc(MM_DT)
    in_maps = _prep_in_maps(**inputs)
    res = run_bass_kernel_spmd(
        nc, in_maps, list(range(N_CORES)), trace=trace, tmpdir=tmpdir
    )
    return _unprep_output(res.results), res.exec_time_ns


def kernel(**inputs) -> np.ndarray:
    out, _ = _run(inputs, trace=False)
    return out
